# revision 6
# baseline (speedup 1.0000x reference)
"""Trainium2 Bass kernel for GNN message passing:

    h = segment_sum(x[src] * (edge_basis @ W.T + b), dst, num_segments=N)

Strategy (node-sharded, sort-based; no collectives — each core owns its
output rows exclusively):
  - Host: sort edges by dst; core c owns the contiguous node range
    [c*N/8, (c+1)*N/8). Within a core, nodes are grouped into blocks of
    128; each block's (contiguous, because sorted) edge list is padded to
    tiles of 128 edges. Blocks are processed in per-core descending-size
    order so the shared (SPMD-identical) tile schedule T_list[j] =
    max_over_cores(j-th largest block) stays tight (~3% padding); the
    host un-permutes the output blocks afterwards. Per core the host
    materializes one packed stream (single DMA per block, issued on
    alternating HWDGE rings):
      * st  [128, TT*192] bf16 : per block, edge_basis tiles TRANSPOSED
        ([r, 128e], consumed as matmul weights) followed by x[src]
        gathered rows ([128e, 64], edge on partition).
      * rel [128, TT]     f32  : dst - block_base per edge (0..127).
      * wt  [128, 64]     bf16 : W.T (matmul rhs, resident).
    Pad edges have eb = 0, xs = 0, rel = 0 -> contribute exactly 0.
  - Device per tile (128 edges):
      PE:  filt[128e,64d](psum) = ebT_tile.T @ WT   (per-tile weight swap)
      ACT: filt -> bf16 SBUF (chunk-batched copy)   (+ bias if b != 0)
      DVE: m = xs * filt (bf16, chunk-batched)
      DVE: S[128e,128n] = one_hot(rel) via iota + is_equal (bf16, exact)
      PE:  psum_h[128n,64d] += S.T @ m              (accumulate per block)
    Per block: ACT copies psum_h into a resident SBUF strip; one DMA at
    the end stores h [128, 49*64] f32; host de-interleaves to [N/8, 64].
  Accuracy: inputs quantized to bf16, accumulation in f32 PSUM; measured
  rel RMS error ~3.5e-3 (resid_var ~1.2e-5) vs the f32 reference.
"""

import math
from contextlib import ExitStack

import numpy as np
import ml_dtypes

import concourse.bass as bass
import concourse.bacc as bacc
import concourse.tile as tile
from concourse import mybir
from concourse.bass_utils import run_bass_kernel_spmd

BF16 = ml_dtypes.bfloat16

# Problem configuration (hardcoded per the task spec).
N_NODES = 50000
N_EDGES = 800000
D_IN = 64
D_RADIAL = 128
N_CORES = 8

LAST_BUILD = None  # (nc, in_maps) of the most recent build, for test harnesses

BLK = 128          # nodes per block (= one-hot width = psum partition dim)
CHUNK_MAX = 8     # max tiles per DVE/ACT batch (psum_filt <= 2 banks)
S_DTYPE = "bf16"     # one-hot dtype: "f8" (FWL 4x weight load) or "bf16"
S_ENGINE = "vector"  # build one-hot on: "gpsimd" (offload DVE) or "vector"
S_POOL_MOD = 3       # every k-th one-hot built on gpsimd (0 = never)
DMA_GROUP = 2        # stream blocks fetched per DMA
SORT_BLOCKS = True   # per-core descending-size block order (less padding)
MODE = "full"        # "full" | "dma" (stream DMAs only) | "compute" (no DMA)
DMA_QUEUES = 2       # stream DMA issue queues: cycle over sync,scalar,vector,tensor


def _plan(dst_sorted, n_nodes, n_cores):
    """Compute per-(core, block) edge ranges and the shared tile schedule.

    Returns (T_list, e_start, e_end, npc, n_blocks):
      T_list[j]  = tiles allocated for local block j (same for all cores)
      e_start/e_end[c, j] = edge index range (into the sorted edge order)
    """
    npc = n_nodes // n_cores
    assert npc * n_cores == n_nodes
    n_blocks = math.ceil(npc / BLK)
    bounds = np.empty((n_cores, n_blocks + 1), np.int64)
    for c in range(n_cores):
        for j in range(n_blocks + 1):
            bounds[c, j] = c * npc + min(j * BLK, npc)
    e_bounds = np.searchsorted(dst_sorted, bounds.ravel()).reshape(bounds.shape)
    e_start = e_bounds[:, :-1]
    e_end = e_bounds[:, 1:]
    counts = e_end - e_start
    tiles_needed = np.maximum((counts + BLK - 1) // BLK, 1)
    if SORT_BLOCKS:
        # Each core processes its blocks in descending-size order; loop
        # position j is sized by the max over cores of the j-th largest
        # block, which is much tighter than max over cores per block.
        perm = np.argsort(-tiles_needed, axis=1, kind="stable")
        sorted_tiles = np.take_along_axis(tiles_needed, perm, axis=1)
        T_list = sorted_tiles.max(axis=0)
    else:
        perm = np.tile(np.arange(n_blocks), (n_cores, 1))
        T_list = tiles_needed.max(axis=0)
    return T_list, e_start, e_end, npc, n_blocks, perm


def _prepare_core(eb_bf, srcx, order, e_start, e_end, T_list, perm,
                  core, npc, n_blocks, d_in, d_radial):
    """Build the per-core device input arrays (see module docstring)."""
    TT = int(T_list.sum())
    idx = np.full(TT * BLK, -1, np.int64)  # into sorted-edge order
    off = 0
    for j in range(n_blocks):
        blk = int(perm[core, j])
        s, e = e_start[core, blk], e_end[core, blk]
        idx[off * BLK: off * BLK + (e - s)] = order[s:e]
        off += int(T_list[j])
    pad = idx < 0
    idxc = np.where(pad, 0, idx)

    # ebT tiles: [TT, 128e, d_radial] -> [d_radial, TT, 128e]
    ebg = eb_bf[idxc]
    ebg[pad] = 0
    ebT = ebg.reshape(TT, BLK, d_radial).transpose(2, 0, 1)

    # xs tiles: [TT, 128e, d_in] -> [128e, TT, d_in]
    xsg = srcx[idxc]                  # x[src] per original edge id
    xsg[pad] = 0
    xs = xsg.reshape(TT, BLK, d_in).transpose(1, 0, 2)

    # Pack per block: stream[:, 192*off : 192*off+T*128] = ebT block,
    #                 stream[:, 192*off+T*128 : 192*(off+T)] = xs block.
    stream = np.empty((BLK, TT * (BLK + d_in)), eb_bf.dtype)
    off = 0
    for j in range(len(T_list)):
        T = int(T_list[j])
        c0 = off * (BLK + d_in)
        stream[:, c0:c0 + T * BLK] = ebT[:, off:off + T].reshape(BLK, T * BLK)
        stream[:, c0 + T * BLK:c0 + T * (BLK + d_in)] = \
            xs[:, off:off + T].reshape(BLK, T * d_in)
        off += T
    return stream, idx, pad


def build_program(TT, T_list, n_blocks, d_in=D_IN, d_radial=D_RADIAL,
                  n_cores=N_CORES, has_bias=False, repeat=1, loop_n=0):
    """Build + compile the SPMD Bass program (identical across cores)."""
    nc = bacc.Bacc("TRN2", target_bir_lowering=False, debug=False,
                   num_devices=n_cores)
    bf = mybir.dt.bfloat16
    f32 = mybir.dt.float32
    s_dt = mybir.dt.float8e4 if S_DTYPE == "f8" else bf

    st_d = nc.dram_tensor("st", [BLK, TT * (BLK + d_in)], bf,
                          kind="ExternalInput")
    rel_d = nc.dram_tensor("rel", [BLK, TT], f32, kind="ExternalInput")
    wt_d = nc.dram_tensor("wt", [d_radial, d_in], bf, kind="ExternalInput")
    if has_bias:
        bb_d = nc.dram_tensor("bb", [BLK, CHUNK_MAX * d_in], bf,
                              kind="ExternalInput")
    h_d = nc.dram_tensor("h", [BLK, n_blocks * d_in], f32, kind="ExternalOutput")

    T_cap = int(max(T_list))

    with TileContextCompat(nc) as tc, ExitStack() as ctx:
        const = ctx.enter_context(tc.tile_pool(name="const", bufs=1))
        ebp = ctx.enter_context(tc.tile_pool(name="ebp", bufs=4))
        fsb = ctx.enter_context(tc.tile_pool(name="fsb", bufs=4))
        msb = ctx.enter_context(tc.tile_pool(name="msb", bufs=5))
        ssb = ctx.enter_context(tc.tile_pool(name="ssb", bufs=24))
        pfil = ctx.enter_context(
            tc.tile_pool(name="pfil", bufs=3, space="PSUM"))
        ph = ctx.enter_context(tc.tile_pool(name="ph", bufs=3, space="PSUM"))

        wt_t = const.tile([d_radial, d_in], bf)
        nc.sync.dma_start(wt_t[:], wt_d.ap())
        rel_all = const.tile([BLK, TT], f32)
        nc.sync.dma_start(rel_all[:], rel_d.ap())
        if has_bias:
            bb_t = const.tile([BLK, CHUNK_MAX * d_in], bf)
            nc.sync.dma_start(bb_t[:], bb_d.ap())
        iota_i = const.tile([BLK, BLK], mybir.dt.int32)
        nc.gpsimd.iota(iota_i[:], pattern=[[1, BLK]], base=0,
                       channel_multiplier=0)
        iota_bf = const.tile([BLK, BLK], bf)
        nc.vector.tensor_copy(iota_bf[:], iota_i[:])
        h_all = const.tile([BLK, n_blocks * d_in], f32)

        import contextlib
        loop_cm = (tc.For_i(0, loop_n, 1,
                            hint_engines=(mybir.EngineType.PE,
                                          mybir.EngineType.DVE,
                                          mybir.EngineType.Activation,
                                          mybir.EngineType.SP),
                            staggered_reset=True)
                   if loop_n else contextlib.nullcontext())
        # Group consecutive blocks into one stream DMA each.
        grp = max(1, DMA_GROUP)
        groups = [list(range(j, min(j + grp, n_blocks)))
                  for j in range(0, n_blocks, grp)]
        g_caps = [sum(int(T_list[j]) for j in g) for g in groups]
        G_cap = max(g_caps)

        with loop_cm:
          for _rep in range(repeat):
            # One-chunk-skewed software pipeline: scatter matmuls of chunk
            # i are emitted after the filt matmuls of chunk i+1, so the
            # in-order PE always has independent filt work to run while
            # chunk i's ACT-copy + DVE-multiply latency drains.
            pending = []  # queued scatter jobs

            def emit_scatter(job):
                (s_tiles, m_sb, base, cs, psum_h, Tj, j_blk) = job
                for k in range(cs):
                    t = base + k
                    nc.tensor.matmul(psum_h[:], s_tiles[k][:],
                                     m_sb[:, k * d_in:(k + 1) * d_in],
                                     start=(t == 0), stop=(t == Tj - 1))
                if base + cs == Tj:  # block finished -> drain to SBUF strip
                    nc.scalar.copy(h_all[:, j_blk * d_in:(j_blk + 1) * d_in],
                                   psum_h[:])

            dq = [nc.sync, nc.scalar, nc.vector, nc.tensor][:max(1, DMA_QUEUES)]
            off = 0
            for gi, g in enumerate(groups):
                Tg = g_caps[gi]
                st_t = ebp.tile([BLK, G_cap * (BLK + d_in)], bf, tag="st")
                dma_eng = dq[gi % len(dq)]
                c0 = off * (BLK + d_in)
                if MODE != "compute":
                    dma_eng.dma_start(st_t[:, :Tg * (BLK + d_in)],
                                      st_d.ap()[:, c0:c0 + Tg * (BLK + d_in)])
                else:
                    # tiny DMA to register a write dep on the tile
                    dma_eng.dma_start(st_t[:, :BLK + d_in],
                                      st_d.ap()[:, c0:c0 + BLK + d_in])
                if MODE == "dma":
                    off += Tg
                    continue
                loff = 0  # tile offset within the group
                for j in g:
                    Tj = int(T_list[j])
                    blk_t = st_t[:, loff * (BLK + d_in):
                                 (loff + Tj) * (BLK + d_in)]
                    eb_t = blk_t[:, :Tj * BLK]
                    xs_t = blk_t[:, Tj * BLK:Tj * (BLK + d_in)]
                    psum_h = ph.tile([BLK, d_in], f32)

                    n_chunks = math.ceil(Tj / CHUNK_MAX)
                    chunk = math.ceil(Tj / n_chunks)
                    base = 0
                    while base < Tj:
                        cs = min(chunk, Tj - base)
                        s_tiles = []
                        for k in range(cs):
                            t = base + k
                            gt = off + loff + t  # global tile index
                            s_eng = (nc.gpsimd if (S_POOL_MOD and
                                     gt % S_POOL_MOD == S_POOL_MOD - 1)
                                     else nc.vector)
                            s_t = ssb.tile([BLK, BLK], s_dt, tag="s")
                            s_eng.tensor_scalar(
                                s_t[:], iota_bf[:],
                                rel_all[:, off + loff + t:
                                        off + loff + t + 1], None,
                                op0=mybir.AluOpType.is_equal)
                            s_tiles.append(s_t)
                        pf = pfil.tile([BLK, CHUNK_MAX * d_in], f32, tag="pf")
                        for k in range(cs):
                            t = base + k
                            nc.tensor.matmul(pf[:, k * d_in:(k + 1) * d_in],
                                             eb_t[:, t * BLK:(t + 1) * BLK],
                                             wt_t[:], start=True, stop=True)
                        f_sb = fsb.tile([BLK, CHUNK_MAX * d_in], bf, tag="f")
                        nc.scalar.copy(f_sb[:, :cs * d_in], pf[:, :cs * d_in])
                        if has_bias:
                            nc.vector.tensor_add(f_sb[:, :cs * d_in],
                                                 f_sb[:, :cs * d_in],
                                                 bb_t[:, :cs * d_in])
                        m_sb = msb.tile([BLK, CHUNK_MAX * d_in], bf, tag="m")
                        nc.vector.tensor_mul(
                            m_sb[:, :cs * d_in],
                            xs_t[:, base * d_in:(base + cs) * d_in],
                            f_sb[:, :cs * d_in])
                        pending.append(
                            (s_tiles, m_sb, base, cs, psum_h, Tj, j))
                        if len(pending) >= 2:
                            emit_scatter(pending.pop(0))
                        base += cs
                    loff += Tj
                off += Tg
            for job in pending:
                emit_scatter(job)
        if MODE != "dma":
            nc.sync.dma_start(h_d.ap(), h_all[:])

    nc.compile()
    return nc


# TileContext wrapper: single place to tweak kwargs if needed.
def TileContextCompat(nc):
    return tile.TileContext(nc)


def _kernel_impl(x, edge_basis, src, dst, W, b,
                 n_nodes, d_in, d_radial, n_cores, run_fn=None):
    dst = np.asarray(dst)
    order = np.argsort(dst, kind="stable")
    dst_sorted = dst[order]
    T_list, e_start, e_end, npc, n_blocks, perm = _plan(dst_sorted, n_nodes,
                                                        n_cores)
    TT = int(T_list.sum())

    eb_bf = np.asarray(edge_basis).astype(BF16)
    srcx = np.asarray(x)[np.asarray(src)].astype(BF16)  # x gathered per edge

    has_bias = bool(np.any(np.asarray(b) != 0))

    in_maps = []
    for c in range(n_cores):
        stream, idx, pad = _prepare_core(
            eb_bf, srcx, order, e_start, e_end, T_list, perm, c, npc,
            n_blocks, d_in, d_radial)
        # rel per slot: node index within the 128-node block; pads -> 0.
        rel_slot = np.zeros(TT * BLK, np.float32)
        valid = ~pad
        rel_slot[valid] = (dst[idx[valid]] - c * npc) % BLK
        rel_arr = np.ascontiguousarray(
            rel_slot.reshape(TT, BLK).T).astype(np.float32)
        m = {
            "st": stream,
            "rel": rel_arr,
            "wt": np.ascontiguousarray(np.asarray(W).T).astype(BF16),
        }
        if has_bias:
            m["bb"] = np.tile(np.asarray(b).astype(BF16), (BLK, CHUNK_MAX))
        in_maps.append(m)

    nc = build_program(TT, T_list, n_blocks, d_in, d_radial, n_cores,
                       has_bias)
    global LAST_BUILD
    LAST_BUILD = (nc, in_maps)
    if run_fn is None:
        res = run_bass_kernel_spmd(nc, in_maps, core_ids=list(range(n_cores)))
        results = res.results
    else:
        results = run_fn(nc, in_maps)

    h = np.empty((n_nodes, d_in), np.float32)
    for c in range(n_cores):
        hc = results[c]["h"].reshape(BLK, n_blocks, d_in).transpose(1, 0, 2)
        blocks = np.empty_like(hc)          # un-permute loop order -> blocks
        blocks[perm[c]] = hc
        blocks = blocks.reshape(n_blocks * BLK, d_in)
        h[c * npc:(c + 1) * npc] = blocks[:npc]
    return h


def kernel(x, edge_basis, src, dst, W, b):
    assert x.shape == (N_NODES, D_IN)
    assert edge_basis.shape == (N_EDGES, D_RADIAL)
    h = _kernel_impl(x, edge_basis, src, dst, W, b,
                     N_NODES, D_IN, D_RADIAL, N_CORES)
    return h.astype(x.dtype)



# revision 10
# speedup vs baseline: 3.5761x; 3.5761x over previous
"""Trainium2 Bass kernel for GNN message passing:

    h = segment_sum(x[src] * (edge_basis @ W.T + b), dst, num_segments=N)

Strategy (node-sharded, slot-aligned; no collectives — each core owns its
output rows exclusively):
  - Host: core c owns nodes [c*N/8, (c+1)*N/8). Within a core, nodes are
    sorted by degree (desc) and grouped into blocks of 128; each node is
    pinned to one SBUF partition. Tile k of a block holds the k-th edge of
    every node in the block (slot-aligned), so a block needs
    Tb = max-degree-in-block tiles; degree-sorted grouping keeps padding
    ~3%. The shared (SPMD-identical) schedule T_list[j] =
    max_over_cores(j-th block's Tb). Slots past a node's degree are pads
    (eb = 0, xs = 0 -> contribute exactly 0). The host materializes one
    packed stream per core (single DMA per block, alternating HWDGE rings):
      * st  [128, TT*192] bf16 : per block, edge_basis tiles TRANSPOSED
        ([r, 128slots], consumed as matmul weights) followed by x[src]
        gathered rows ([128slots, 64], slot on partition).
      * wt  [128, 64]     bf16 : W.T (matmul rhs, resident).
  - Device per tile (128 slots):
      PE:  filt[128s,64d](psum) = ebT_tile.T @ WT   (per-tile weight swap)
      ACT: filt -> bf16 SBUF (chunk-batched copy)   (+ bias if b != 0)
      DVE: m = xs * filt (bf16, chunk-batched)
      PE:  psum_h[128n,64d] += IDENT.T @ m          (slot-aligned scatter:
           constant identity weight, PSUM-accumulated across the block)
    Per block: ACT copies psum_h into a resident SBUF strip; one DMA at
    the end stores h [128, 49*64] f32; host scatters rows back by node id.
  Accuracy: inputs quantized to bf16, accumulation in f32 PSUM; measured
  rel RMS error ~3.5e-3 vs the f32 reference.
"""

import math
from contextlib import ExitStack

import numpy as np
import ml_dtypes

import concourse.bass as bass
import concourse.bacc as bacc
import concourse.tile as tile
from concourse import mybir
from concourse.bass_utils import run_bass_kernel_spmd

BF16 = ml_dtypes.bfloat16

# Problem configuration (hardcoded per the task spec).
N_NODES = 50000
N_EDGES = 800000
D_IN = 64
D_RADIAL = 128
N_CORES = 8

LAST_BUILD = None  # (nc, in_maps) of the most recent build, for test harnesses

BLK = 128          # nodes per block (= psum partition dim)
CHUNK_MAX = 8      # max tiles per DVE/ACT batch (psum_filt <= 1 bank)
MODE = "full"      # "full" | "dma" (stream DMAs only) | "compute" (no DMA)
STAGES = frozenset(["mm1", "copy", "mul", "mm2"])  # compute-stage ablation
DMA_QUEUES = 2     # stream DMA issue queues: cycle over sync,scalar,vector,tensor
DMA_GROUP = 1      # stream blocks fetched per DMA


def _plan(dst_sorted, n_nodes, n_cores):
    """Slot-aligned plan from the dst-sorted edge order.

    Returns (T_list, node_ids, deg, npc, n_blocks, starts):
      T_list[j]        = tiles allocated for local block j (same for all
                         cores; per-core blocks are in descending-Tb order)
      node_ids[c,j,p]  = global node id owning (block j, partition p)
      deg[n]           = edge count of global node n
      starts[n]        = first index of node n's edge run in dst-sorted order
    """
    npc = n_nodes // n_cores
    assert npc * n_cores == n_nodes
    n_blocks = math.ceil(npc / BLK)
    deg = np.zeros(n_nodes + 1, np.int64)
    np.add.at(deg, dst_sorted, 1)
    starts = np.zeros(n_nodes + 1, np.int64)
    np.cumsum(deg[:-1], out=starts[1:])
    deg = deg[:-1]

    node_ids = np.full((n_cores, n_blocks, BLK), -1, np.int64)
    Tb = np.empty((n_cores, n_blocks), np.int64)
    for c in range(n_cores):
        nodes = np.arange(c * npc, (c + 1) * npc)
        order = np.argsort(-deg[nodes], kind="stable")
        nodes = nodes[order]
        pad = n_blocks * BLK - npc
        ids = np.concatenate([nodes, np.full(pad, -1, np.int64)])
        node_ids[c] = ids.reshape(n_blocks, BLK)
        d = np.where(node_ids[c] >= 0, deg[np.maximum(node_ids[c], 0)], 0)
        Tb[c] = np.maximum(d.max(axis=1), 1)
    T_list = Tb.max(axis=0)
    return T_list, node_ids, deg, npc, n_blocks, starts


def _prepare_core(eb_bf, srcx, order, T_list, node_ids, deg, starts,
                  core, n_blocks, d_in, d_radial):
    """Build the per-core packed stream (see module docstring)."""
    TT = int(T_list.sum())
    idx = np.full((TT, BLK), -1, np.int64)  # into dst-sorted edge order
    off = 0
    for j in range(n_blocks):
        T = int(T_list[j])
        ids = node_ids[core, j]                    # [BLK]
        valid = ids >= 0
        d = np.where(valid, deg[np.maximum(ids, 0)], 0)
        s = np.where(valid, starts[np.maximum(ids, 0)], 0)
        k = np.arange(T)[:, None]                  # [T, 1]
        sl = k < d[None, :]
        idx[off:off + T][sl] = (s[None, :] + k)[sl]
        off += T
    pad = idx < 0
    idxc = np.where(pad, 0, order[np.where(pad, 0, idx)])
    idxc = np.where(pad, 0, idxc)

    # ebT tiles: [TT, 128s, d_radial] -> [d_radial, TT, 128s]
    ebg = eb_bf[idxc]
    ebg[pad] = 0
    ebT = ebg.transpose(2, 0, 1)

    # xs tiles: [TT, 128s, d_in] -> [128s, TT, d_in]
    xsg = srcx[idxc]                  # x[src] per original edge id
    xsg[pad] = 0
    xs = xsg.transpose(1, 0, 2)

    # Pack per block: stream[:, 192*off : 192*off+T*128] = ebT block,
    #                 stream[:, 192*off+T*128 : 192*(off+T)] = xs block.
    stream = np.empty((BLK, TT * (BLK + d_in)), eb_bf.dtype)
    off = 0
    for j in range(len(T_list)):
        T = int(T_list[j])
        c0 = off * (BLK + d_in)
        stream[:, c0:c0 + T * BLK] = ebT[:, off:off + T].reshape(BLK, T * BLK)
        stream[:, c0 + T * BLK:c0 + T * (BLK + d_in)] = \
            xs[:, off:off + T].reshape(BLK, T * d_in)
        off += T
    return stream


def build_program(TT, T_list, n_blocks, d_in=D_IN, d_radial=D_RADIAL,
                  n_cores=N_CORES, has_bias=False, repeat=1, loop_n=0):
    """Build + compile the SPMD Bass program (identical across cores)."""
    nc = bacc.Bacc("TRN2", target_bir_lowering=False, debug=False,
                   num_devices=n_cores)
    bf = mybir.dt.bfloat16
    f32 = mybir.dt.float32

    st_d = nc.dram_tensor("st", [BLK, TT * (BLK + d_in)], bf,
                          kind="ExternalInput")
    wt_d = nc.dram_tensor("wt", [d_radial, d_in], bf, kind="ExternalInput")
    if has_bias:
        bb_d = nc.dram_tensor("bb", [BLK, CHUNK_MAX * d_in], bf,
                              kind="ExternalInput")
    h_d = nc.dram_tensor("h", [BLK, n_blocks * d_in], f32, kind="ExternalOutput")

    with TileContextCompat(nc) as tc, ExitStack() as ctx:
        const = ctx.enter_context(tc.tile_pool(name="const", bufs=1))
        ebp = ctx.enter_context(tc.tile_pool(name="ebp", bufs=4))
        fsb = ctx.enter_context(tc.tile_pool(name="fsb", bufs=4))
        msb = ctx.enter_context(tc.tile_pool(name="msb", bufs=5))
        pfil = ctx.enter_context(
            tc.tile_pool(name="pfil", bufs=3, space="PSUM"))
        ph = ctx.enter_context(tc.tile_pool(name="ph", bufs=3, space="PSUM"))

        wt_t = const.tile([d_radial, d_in], bf)
        nc.sync.dma_start(wt_t[:], wt_d.ap())
        if has_bias:
            bb_t = const.tile([BLK, CHUNK_MAX * d_in], bf)
            nc.sync.dma_start(bb_t[:], bb_d.ap())
        # Constant identity [128, 128] bf16: ident[p, f] = (f == p).
        iota_i = const.tile([BLK, BLK], mybir.dt.int32)
        nc.gpsimd.iota(iota_i[:], pattern=[[1, BLK]], base=0,
                       channel_multiplier=0)
        iota_bf = const.tile([BLK, BLK], bf)
        nc.vector.tensor_copy(iota_bf[:], iota_i[:])
        iota_col_i = const.tile([BLK, 1], mybir.dt.int32)
        nc.gpsimd.iota(iota_col_i[:], pattern=[[1, 1]], base=0,
                       channel_multiplier=1)
        iota_col = const.tile([BLK, 1], f32)
        nc.vector.tensor_copy(iota_col[:], iota_col_i[:])
        ident = const.tile([BLK, BLK], bf)
        nc.vector.tensor_scalar(ident[:], iota_bf[:], iota_col[:], None,
                                op0=mybir.AluOpType.is_equal)
        h_all = const.tile([BLK, n_blocks * d_in], f32)

        # Group consecutive blocks into one stream DMA each.
        grp = max(1, DMA_GROUP)
        groups = [list(range(j, min(j + grp, n_blocks)))
                  for j in range(0, n_blocks, grp)]
        g_caps = [sum(int(T_list[j]) for j in g) for g in groups]
        G_cap = max(g_caps)

        import contextlib
        loop_cm = (tc.For_i(0, loop_n, 1,
                            hint_engines=(mybir.EngineType.PE,
                                          mybir.EngineType.DVE,
                                          mybir.EngineType.Activation,
                                          mybir.EngineType.SP),
                            staggered_reset=True)
                   if loop_n else contextlib.nullcontext())
        with loop_cm:
          for _rep in range(repeat):
            # One-chunk-skewed software pipeline: scatter matmuls of chunk
            # i are emitted after the filt matmuls of chunk i+1, so the
            # in-order PE always has independent filt work to run while
            # chunk i's ACT-copy + DVE-multiply latency drains.
            pending = []  # queued scatter jobs

            def emit_scatter(job):
                (m_sb, base, cs, psum_h, Tj, j_blk) = job
                for k in range(cs):
                    t = base + k
                    nc.tensor.matmul(psum_h[:], ident[:],
                                     m_sb[:, k * d_in:(k + 1) * d_in],
                                     start=(t == 0), stop=(t == Tj - 1))
                if base + cs == Tj:  # block finished -> drain to SBUF strip
                    nc.scalar.copy(h_all[:, j_blk * d_in:(j_blk + 1) * d_in],
                                   psum_h[:])

            dq = [nc.sync, nc.scalar, nc.vector, nc.tensor][:max(1, DMA_QUEUES)]
            off = 0
            for gi, g in enumerate(groups):
                Tg = g_caps[gi]
                st_t = ebp.tile([BLK, G_cap * (BLK + d_in)], bf, tag="st")
                dma_eng = dq[gi % len(dq)]
                c0 = off * (BLK + d_in)
                if MODE != "compute":
                    dma_eng.dma_start(st_t[:, :Tg * (BLK + d_in)],
                                      st_d.ap()[:, c0:c0 + Tg * (BLK + d_in)])
                else:
                    # tiny DMA to register a write dep on the tile
                    dma_eng.dma_start(st_t[:, :BLK + d_in],
                                      st_d.ap()[:, c0:c0 + BLK + d_in])
                if MODE == "dma":
                    off += Tg
                    continue
                loff = 0  # tile offset within the group
                for j in g:
                    Tj = int(T_list[j])
                    blk_t = st_t[:, loff * (BLK + d_in):
                                 (loff + Tj) * (BLK + d_in)]
                    eb_t = blk_t[:, :Tj * BLK]
                    xs_t = blk_t[:, Tj * BLK:Tj * (BLK + d_in)]
                    psum_h = ph.tile([BLK, d_in], f32)

                    n_chunks = math.ceil(Tj / CHUNK_MAX)
                    chunk = math.ceil(Tj / n_chunks)
                    base = 0
                    while base < Tj:
                        cs = min(chunk, Tj - base)
                        pf = pfil.tile([BLK, CHUNK_MAX * d_in], f32, tag="pf")
                        if "mm1" in STAGES:
                            for k in range(cs):
                                t = base + k
                                nc.tensor.matmul(
                                    pf[:, k * d_in:(k + 1) * d_in],
                                    eb_t[:, t * BLK:(t + 1) * BLK],
                                    wt_t[:], start=True, stop=True)
                        f_sb = fsb.tile([BLK, CHUNK_MAX * d_in], bf, tag="f")
                        if "copy" in STAGES:
                            nc.scalar.copy(f_sb[:, :cs * d_in],
                                           pf[:, :cs * d_in])
                        if has_bias:
                            nc.vector.tensor_add(f_sb[:, :cs * d_in],
                                                 f_sb[:, :cs * d_in],
                                                 bb_t[:, :cs * d_in])
                        m_sb = msb.tile([BLK, CHUNK_MAX * d_in], bf, tag="m")
                        if "mul" in STAGES:
                            nc.vector.tensor_mul(
                                m_sb[:, :cs * d_in],
                                xs_t[:, base * d_in:(base + cs) * d_in],
                                f_sb[:, :cs * d_in])
                        if "mm2" in STAGES and "mul" in STAGES:
                            pending.append(
                                (m_sb, base, cs, psum_h, Tj, j))
                            if len(pending) >= 2:
                                emit_scatter(pending.pop(0))
                        base += cs
                    loff += Tj
                off += Tg
            for job in pending:
                emit_scatter(job)
        if MODE == "full" and "mm2" in STAGES:
            nc.sync.dma_start(h_d.ap(), h_all[:])

    nc.compile()
    return nc


# TileContext wrapper: single place to tweak kwargs if needed.
def TileContextCompat(nc):
    return tile.TileContext(nc)


def _kernel_impl(x, edge_basis, src, dst, W, b,
                 n_nodes, d_in, d_radial, n_cores, run_fn=None):
    dst = np.asarray(dst)
    order = np.argsort(dst, kind="stable")
    dst_sorted = dst[order]
    T_list, node_ids, deg, npc, n_blocks, starts = _plan(dst_sorted, n_nodes,
                                                         n_cores)
    TT = int(T_list.sum())

    eb_bf = np.asarray(edge_basis).astype(BF16)
    srcx = np.asarray(x)[np.asarray(src)].astype(BF16)  # x gathered per edge

    has_bias = bool(np.any(np.asarray(b) != 0))

    in_maps = []
    for c in range(n_cores):
        stream = _prepare_core(eb_bf, srcx, order, T_list, node_ids, deg,
                               starts, c, n_blocks, d_in, d_radial)
        m = {
            "st": stream,
            "wt": np.ascontiguousarray(np.asarray(W).T).astype(BF16),
        }
        if has_bias:
            m["bb"] = np.tile(np.asarray(b).astype(BF16), (BLK, CHUNK_MAX))
        in_maps.append(m)

    nc = build_program(TT, T_list, n_blocks, d_in, d_radial, n_cores,
                       has_bias)
    global LAST_BUILD
    LAST_BUILD = (nc, in_maps)
    if run_fn is None:
        res = run_bass_kernel_spmd(nc, in_maps, core_ids=list(range(n_cores)))
        results = res.results
    else:
        results = run_fn(nc, in_maps)

    h = np.empty((n_nodes, d_in), np.float32)
    for c in range(n_cores):
        hc = results[c]["h"].reshape(BLK, n_blocks, d_in).transpose(1, 0, 2)
        ids = node_ids[c].reshape(-1)
        valid = ids >= 0
        h[ids[valid]] = hc.reshape(n_blocks * BLK, d_in)[valid]
    return h


def kernel(x, edge_basis, src, dst, W, b):
    assert x.shape == (N_NODES, D_IN)
    assert edge_basis.shape == (N_EDGES, D_RADIAL)
    h = _kernel_impl(x, edge_basis, src, dst, W, b,
                     N_NODES, D_IN, D_RADIAL, N_CORES)
    return h.astype(x.dtype)


# revision 15
# speedup vs baseline: 5.0453x; 1.4108x over previous
"""Trainium2 Bass kernel for GNN message passing:

    h = segment_sum(x[src] * (edge_basis @ W.T + b), dst, num_segments=N)

Strategy (node-sharded, slot-aligned; no collectives — each core owns its
output rows exclusively):
  - Host: core c owns nodes [c*N/8, (c+1)*N/8). Within a core, nodes are
    sorted by degree (desc) and grouped into blocks of 128; each node is
    pinned to one SBUF partition. Tile k of a block holds the k-th edge of
    every node in the block (slot-aligned), so a block needs
    Tb = max-degree-in-block tiles; degree-sorted grouping keeps padding
    ~3%. The shared (SPMD-identical) schedule T_list[j] =
    max_over_cores(j-th block's Tb). Slots past a node's degree are pads
    (eb = 0, xs = 0 -> contribute exactly 0). The host materializes one
    packed stream per core (single DMA per block, alternating HWDGE rings):
      * st  [128, TT*192] bf16 : per block, edge_basis tiles TRANSPOSED
        ([r, 128slots], consumed as matmul weights) followed by x[src]
        gathered rows ([128slots, 64], slot on partition).
      * wt  [128, 64]     bf16 : W.T (matmul rhs, resident).
  - Device per tile (128 slots):
      PE:  filt[128s,64d](psum) = ebT_tile.T @ WT   (per-tile weight swap)
      ACT: filt -> bf16 SBUF (chunk-batched copy)   (+ bias if b != 0)
      DVE: m = xs * filt (bf16, chunk-batched)
      PE:  psum_h[128n,64d] += IDENT.T @ m          (slot-aligned scatter:
           constant identity weight, PSUM-accumulated across the block)
    Per block: ACT copies psum_h into a resident SBUF strip; one DMA at
    the end stores h [128, 49*64] f32; host scatters rows back by node id.
  Accuracy: inputs quantized to bf16, accumulation in f32 PSUM; measured
  rel RMS error ~3.5e-3 vs the f32 reference.
"""

import math
from contextlib import ExitStack

import numpy as np
import ml_dtypes

import concourse.bass as bass
import concourse.bacc as bacc
import concourse.tile as tile
from concourse import mybir
from concourse.bass_utils import run_bass_kernel_spmd

BF16 = ml_dtypes.bfloat16

# Problem configuration (hardcoded per the task spec).
N_NODES = 50000
N_EDGES = 800000
D_IN = 64
D_RADIAL = 128
N_CORES = 8

LAST_BUILD = None  # (nc, in_maps) of the most recent build, for test harnesses

BLK = 128          # nodes per block (= psum partition dim)
CHUNK_MAX = 8      # max tiles per DVE/ACT batch (psum_filt <= 1 bank)
SKEW = 4           # software-pipeline depth (chunks between mm1 and mm2)
COPY_SPLIT = True  # alternate filt psum->SBUF copies between ACT and DVE
MODE = "full"      # "full" | "dma" (stream DMAs only) | "compute" (no DMA)
STAGES = frozenset(["mm1", "copy", "mul", "mm2"])  # compute-stage ablation
DMA_QUEUES = 2     # stream DMA issue queues: cycle over sync,scalar,vector,tensor
DMA_GROUP = 1      # stream blocks fetched per DMA


def _plan(dst_sorted, n_nodes, n_cores):
    """Slot-aligned plan from the dst-sorted edge order.

    Returns (T_list, node_ids, deg, npc, n_blocks, starts):
      T_list[j]        = tiles allocated for local block j (same for all
                         cores; per-core blocks are in descending-Tb order)
      node_ids[c,j,p]  = global node id owning (block j, partition p)
      deg[n]           = edge count of global node n
      starts[n]        = first index of node n's edge run in dst-sorted order
    """
    npc = n_nodes // n_cores
    assert npc * n_cores == n_nodes
    n_blocks = math.ceil(npc / BLK)
    deg = np.zeros(n_nodes + 1, np.int64)
    np.add.at(deg, dst_sorted, 1)
    starts = np.zeros(n_nodes + 1, np.int64)
    np.cumsum(deg[:-1], out=starts[1:])
    deg = deg[:-1]

    node_ids = np.full((n_cores, n_blocks, BLK), -1, np.int64)
    Tb = np.empty((n_cores, n_blocks), np.int64)
    for c in range(n_cores):
        nodes = np.arange(c * npc, (c + 1) * npc)
        order = np.argsort(-deg[nodes], kind="stable")
        nodes = nodes[order]
        pad = n_blocks * BLK - npc
        ids = np.concatenate([nodes, np.full(pad, -1, np.int64)])
        node_ids[c] = ids.reshape(n_blocks, BLK)
        d = np.where(node_ids[c] >= 0, deg[np.maximum(node_ids[c], 0)], 0)
        Tb[c] = np.maximum(d.max(axis=1), 1)
    T_list = Tb.max(axis=0)
    return T_list, node_ids, deg, npc, n_blocks, starts


def _prepare_core(eb_bf, srcx, order, T_list, node_ids, deg, starts,
                  core, n_blocks, d_in, d_radial):
    """Build the per-core packed stream (see module docstring)."""
    TT = int(T_list.sum())
    idx = np.full((TT, BLK), -1, np.int64)  # into dst-sorted edge order
    off = 0
    for j in range(n_blocks):
        T = int(T_list[j])
        ids = node_ids[core, j]                    # [BLK]
        valid = ids >= 0
        d = np.where(valid, deg[np.maximum(ids, 0)], 0)
        s = np.where(valid, starts[np.maximum(ids, 0)], 0)
        k = np.arange(T)[:, None]                  # [T, 1]
        sl = k < d[None, :]
        idx[off:off + T][sl] = (s[None, :] + k)[sl]
        off += T
    pad = idx < 0
    idxc = np.where(pad, 0, order[np.where(pad, 0, idx)])
    idxc = np.where(pad, 0, idxc)

    # ebT tiles: [TT, 128s, d_radial] -> [d_radial, TT, 128s]
    ebg = eb_bf[idxc]
    ebg[pad] = 0
    ebT = ebg.transpose(2, 0, 1)

    # xs tiles: [TT, 128s, d_in] -> [128s, TT, d_in]
    xsg = srcx[idxc]                  # x[src] per original edge id
    xsg[pad] = 0
    xs = xsg.transpose(1, 0, 2)

    # Pack per block: stream[:, 192*off : 192*off+T*128] = ebT block,
    #                 stream[:, 192*off+T*128 : 192*(off+T)] = xs block.
    stream = np.empty((BLK, TT * (BLK + d_in)), eb_bf.dtype)
    off = 0
    for j in range(len(T_list)):
        T = int(T_list[j])
        c0 = off * (BLK + d_in)
        stream[:, c0:c0 + T * BLK] = ebT[:, off:off + T].reshape(BLK, T * BLK)
        stream[:, c0 + T * BLK:c0 + T * (BLK + d_in)] = \
            xs[:, off:off + T].reshape(BLK, T * d_in)
        off += T
    return stream


def build_program(TT, T_list, n_blocks, d_in=D_IN, d_radial=D_RADIAL,
                  n_cores=N_CORES, has_bias=False, repeat=1, loop_n=0):
    """Build + compile the SPMD Bass program (identical across cores)."""
    nc = bacc.Bacc("TRN2", target_bir_lowering=False, debug=False,
                   num_devices=n_cores)
    bf = mybir.dt.bfloat16
    f32 = mybir.dt.float32

    st_d = nc.dram_tensor("st", [BLK, TT * (BLK + d_in)], bf,
                          kind="ExternalInput")
    wt_d = nc.dram_tensor("wt", [d_radial, d_in], bf, kind="ExternalInput")
    if has_bias:
        bb_d = nc.dram_tensor("bb", [BLK, CHUNK_MAX * d_in], bf,
                              kind="ExternalInput")
    h_d = nc.dram_tensor("h", [BLK, n_blocks * d_in], f32, kind="ExternalOutput")

    with TileContextCompat(nc) as tc, ExitStack() as ctx:
        const = ctx.enter_context(tc.tile_pool(name="const", bufs=1))
        ebp = ctx.enter_context(tc.tile_pool(name="ebp", bufs=4))
        fsb = ctx.enter_context(tc.tile_pool(name="fsb", bufs=SKEW + 2))
        msb = ctx.enter_context(tc.tile_pool(name="msb", bufs=SKEW + 2))
        pfil = ctx.enter_context(
            tc.tile_pool(name="pfil", bufs=3, space="PSUM"))
        ph = ctx.enter_context(
            tc.tile_pool(name="ph", bufs=SKEW + 1, space="PSUM"))

        wt_t = const.tile([d_radial, d_in], bf)
        nc.sync.dma_start(wt_t[:], wt_d.ap())
        if has_bias:
            bb_t = const.tile([BLK, CHUNK_MAX * d_in], bf)
            nc.sync.dma_start(bb_t[:], bb_d.ap())
        # Constant identity [128, 128] bf16: ident[p, f] = (f == p).
        iota_i = const.tile([BLK, BLK], mybir.dt.int32)
        nc.gpsimd.iota(iota_i[:], pattern=[[1, BLK]], base=0,
                       channel_multiplier=0)
        iota_bf = const.tile([BLK, BLK], bf)
        nc.vector.tensor_copy(iota_bf[:], iota_i[:])
        iota_col_i = const.tile([BLK, 1], mybir.dt.int32)
        nc.gpsimd.iota(iota_col_i[:], pattern=[[1, 1]], base=0,
                       channel_multiplier=1)
        iota_col = const.tile([BLK, 1], f32)
        nc.vector.tensor_copy(iota_col[:], iota_col_i[:])
        ident = const.tile([BLK, BLK], bf)
        nc.vector.tensor_scalar(ident[:], iota_bf[:], iota_col[:], None,
                                op0=mybir.AluOpType.is_equal)
        h_all = const.tile([BLK, n_blocks * d_in], f32)

        # Group consecutive blocks into one stream DMA each.
        grp = max(1, DMA_GROUP)
        groups = [list(range(j, min(j + grp, n_blocks)))
                  for j in range(0, n_blocks, grp)]
        g_caps = [sum(int(T_list[j]) for j in g) for g in groups]
        G_cap = max(g_caps)

        import contextlib
        loop_cm = (tc.For_i(0, loop_n, 1,
                            hint_engines=(mybir.EngineType.PE,
                                          mybir.EngineType.DVE,
                                          mybir.EngineType.Activation,
                                          mybir.EngineType.SP),
                            staggered_reset=True)
                   if loop_n else contextlib.nullcontext())
        with loop_cm:
          for _rep in range(repeat):
            # One-chunk-skewed software pipeline: scatter matmuls of chunk
            # i are emitted after the filt matmuls of chunk i+1, so the
            # in-order PE always has independent filt work to run while
            # chunk i's ACT-copy + DVE-multiply latency drains.
            pending = []  # queued scatter jobs

            def emit_scatter(job):
                (m_sb, base, cs, psum_h, Tj, j_blk) = job
                for k in range(cs):
                    t = base + k
                    nc.tensor.matmul(psum_h[:], ident[:],
                                     m_sb[:, k * d_in:(k + 1) * d_in],
                                     start=(t == 0), stop=(t == Tj - 1))
                if base + cs == Tj:  # block finished -> drain to SBUF strip
                    nc.scalar.copy(h_all[:, j_blk * d_in:(j_blk + 1) * d_in],
                                   psum_h[:])

            dq = [nc.sync, nc.scalar, nc.vector, nc.tensor][:max(1, DMA_QUEUES)]
            off = 0
            chunk_no = 0
            for gi, g in enumerate(groups):
                Tg = g_caps[gi]
                st_t = ebp.tile([BLK, G_cap * (BLK + d_in)], bf, tag="st")
                dma_eng = dq[gi % len(dq)]
                c0 = off * (BLK + d_in)
                if MODE != "compute":
                    dma_eng.dma_start(st_t[:, :Tg * (BLK + d_in)],
                                      st_d.ap()[:, c0:c0 + Tg * (BLK + d_in)])
                else:
                    # tiny DMA to register a write dep on the tile
                    dma_eng.dma_start(st_t[:, :BLK + d_in],
                                      st_d.ap()[:, c0:c0 + BLK + d_in])
                if MODE == "dma":
                    off += Tg
                    continue
                loff = 0  # tile offset within the group
                for j in g:
                    Tj = int(T_list[j])
                    blk_t = st_t[:, loff * (BLK + d_in):
                                 (loff + Tj) * (BLK + d_in)]
                    eb_t = blk_t[:, :Tj * BLK]
                    xs_t = blk_t[:, Tj * BLK:Tj * (BLK + d_in)]
                    psum_h = ph.tile([BLK, d_in], f32)

                    n_chunks = math.ceil(Tj / CHUNK_MAX)
                    chunk = math.ceil(Tj / n_chunks)
                    base = 0
                    while base < Tj:
                        cs = min(chunk, Tj - base)
                        pf = pfil.tile([BLK, CHUNK_MAX * d_in], f32, tag="pf")
                        if "mm1" in STAGES:
                            for k in range(cs):
                                t = base + k
                                nc.tensor.matmul(
                                    pf[:, k * d_in:(k + 1) * d_in],
                                    eb_t[:, t * BLK:(t + 1) * BLK],
                                    wt_t[:], start=True, stop=True)
                        f_sb = fsb.tile([BLK, CHUNK_MAX * d_in], bf, tag="f")
                        if "copy" in STAGES:
                            if COPY_SPLIT and (chunk_no % 2 == 1):
                                nc.vector.tensor_copy(f_sb[:, :cs * d_in],
                                                      pf[:, :cs * d_in])
                            else:
                                nc.scalar.copy(f_sb[:, :cs * d_in],
                                               pf[:, :cs * d_in])
                        if has_bias:
                            nc.vector.tensor_add(f_sb[:, :cs * d_in],
                                                 f_sb[:, :cs * d_in],
                                                 bb_t[:, :cs * d_in])
                        m_sb = msb.tile([BLK, CHUNK_MAX * d_in], bf, tag="m")
                        if "mul" in STAGES:
                            nc.vector.tensor_mul(
                                m_sb[:, :cs * d_in],
                                xs_t[:, base * d_in:(base + cs) * d_in],
                                f_sb[:, :cs * d_in])
                        if "mm2" in STAGES and "mul" in STAGES:
                            pending.append(
                                (m_sb, base, cs, psum_h, Tj, j))
                            if len(pending) >= SKEW:
                                emit_scatter(pending.pop(0))
                        base += cs
                        chunk_no += 1
                    loff += Tj
                off += Tg
            for job in pending:
                emit_scatter(job)
        if MODE == "full" and "mm2" in STAGES:
            nc.sync.dma_start(h_d.ap(), h_all[:])

    nc.compile()
    return nc


# TileContext wrapper: single place to tweak kwargs if needed.
def TileContextCompat(nc):
    return tile.TileContext(nc)


def _kernel_impl(x, edge_basis, src, dst, W, b,
                 n_nodes, d_in, d_radial, n_cores, run_fn=None):
    dst = np.asarray(dst)
    order = np.argsort(dst, kind="stable")
    dst_sorted = dst[order]
    T_list, node_ids, deg, npc, n_blocks, starts = _plan(dst_sorted, n_nodes,
                                                         n_cores)
    TT = int(T_list.sum())

    eb_bf = np.asarray(edge_basis).astype(BF16)
    srcx = np.asarray(x)[np.asarray(src)].astype(BF16)  # x gathered per edge

    has_bias = bool(np.any(np.asarray(b) != 0))

    in_maps = []
    for c in range(n_cores):
        stream = _prepare_core(eb_bf, srcx, order, T_list, node_ids, deg,
                               starts, c, n_blocks, d_in, d_radial)
        m = {
            "st": stream,
            "wt": np.ascontiguousarray(np.asarray(W).T).astype(BF16),
        }
        if has_bias:
            m["bb"] = np.tile(np.asarray(b).astype(BF16), (BLK, CHUNK_MAX))
        in_maps.append(m)

    nc = build_program(TT, T_list, n_blocks, d_in, d_radial, n_cores,
                       has_bias)
    global LAST_BUILD
    LAST_BUILD = (nc, in_maps)
    if run_fn is None:
        res = run_bass_kernel_spmd(nc, in_maps, core_ids=list(range(n_cores)))
        results = res.results
    else:
        results = run_fn(nc, in_maps)

    h = np.empty((n_nodes, d_in), np.float32)
    for c in range(n_cores):
        hc = results[c]["h"].reshape(BLK, n_blocks, d_in).transpose(1, 0, 2)
        ids = node_ids[c].reshape(-1)
        valid = ids >= 0
        h[ids[valid]] = hc.reshape(n_blocks * BLK, d_in)[valid]
    return h


def kernel(x, edge_basis, src, dst, W, b):
    assert x.shape == (N_NODES, D_IN)
    assert edge_basis.shape == (N_EDGES, D_RADIAL)
    h = _kernel_impl(x, edge_basis, src, dst, W, b,
                     N_NODES, D_IN, D_RADIAL, N_CORES)
    return h.astype(x.dtype)
